# revision 20
# baseline (speedup 1.0000x reference)
"""Trainium2 Bass kernel for a 2-layer BiLSTM with legacy softmax-over-batch
attention (nn_BILSTM_withAttention2layer).

Sharding: data-parallel over batch B=64 across 8 NeuronCores (8 batches per
core). All weights replicated on device. The legacy softmax over the *batch*
axis in both attention blocks is handled with on-device collectives:
  - AllReduce(add) of per-core exp-sums for the prefix-attention denominators
  - AllReduce(add) of per-core exp-sums for the two full-attention softmaxes
  - AllGather of the per-direction final hidden states (the torch-faithful
    h_n.view(B, 2H) mixes batches, so every core needs other cores' finals)

Host->device I/O is latency/bandwidth-dominated (axon tunnel), so the wrapper
ships the minimum bytes per call:
  - all weights + embedding table in ONE int8 blob (symmetric per-row quant,
    f32 scales in a parallel blob), sharded 1/8 per core and AllGathered +
    dequantized on device
  - per-core token indices only ([16, BL*T/16] u16, replicated to the 8
    partition groups on device)
  - ident / inv-prefix-count / ones constants generated on device
    (iota + affine_select) instead of shipped

Layouts (per core, bl = 8 local batches):
  - time-major "T" tensors [128, bl*T] with column  b*T + t
  - LSTM state/gates kept as [H=128 partitions, (gate,dir,b) free]
  - gates PSUM bank [128, 64]: col (2g+d)*8 + b, gate order (i, f, o, g)
  - xg (input projections) precomputed as bf16 [128, 8*bl*T], chunk (2g+d);
    backward-direction chunks stored time-reversed so the recurrence reads
    a uniform forward index.
"""

import os
import dataclasses
import numpy as np

import concourse.bass as bass
import concourse.mybir as mybir
import concourse.tile as tile
from concourse import bacc
from concourse import bass_utils

F32 = mybir.dt.float32
BF16 = mybir.dt.bfloat16
U16 = mybir.dt.uint16
AF = mybir.ActivationFunctionType
ALU = mybir.AluOpType

H = 128
B = 64
NCORES = 8
BL = B // NCORES  # 8
E = 10
V = 1002

# Packed replicated-weight blob (int8, symmetric per-row quant with f32
# scales in a parallel blob), sharded 1/NCORES per core and AllGathered on
# device. Offsets in elements; row layout mirrors the stored [rows, cols]:
OFF_WXG1 = 0                       # [2][E+1, 4H]
OFF_WHH1 = OFF_WXG1 + 2 * 11 * 512     # [2][H, 4H]
OFF_WXG2 = OFF_WHH1 + 2 * 128 * 512    # [2][2H+1, 4H]
OFF_WHH2 = OFF_WXG2 + 2 * 257 * 512    # [2][H, 4H]
OFF_EMBT = OFF_WHH2 + 2 * 128 * 512    # [E+1, V]
WBLOB_LEN = OFF_EMBT + 11 * V          # 547598
WSH = (WBLOB_LEN + NCORES - 1) // NCORES  # per-core shard (pad to 8*WSH)
SOFF_WXG1 = 0
SOFF_WHH1 = SOFF_WXG1 + 2 * 11
SOFF_WXG2 = SOFF_WHH1 + 2 * 128
SOFF_WHH2 = SOFF_WXG2 + 2 * 257
SOFF_EMBT = SOFF_WHH2 + 2 * 128
SLEN = SOFF_EMBT + 11                  # 1059 scale rows
SSH = (SLEN + NCORES - 1) // NCORES


def _bcast_b(ap2d, nb):
    """[128, N] -> [128, nb, N] with the batch dim broadcast (step 0)."""
    (ps, pc), (fs, fc) = ap2d.ap
    return dataclasses.replace(
        ap2d, ap=[[ps, pc], [0, nb], [fs, fc]]
    )


def build_program(T=512, debug=False):
    nc = bacc.Bacc(
        "TRN2", target_bir_lowering=False, debug=False,
        enable_asserts=False, num_devices=NCORES,
    )
    NBT = BL * T            # flattened (b, t) columns
    PSW = max(T, 128)       # psum big-tile width
    TC = (T + 127) // 128   # t-chunks
    G8 = 8                  # gate-dir chunks (i,f,o,g) x (fwd,bwd)

    # ---------------- DRAM I/O ----------------
    I8 = mybir.dt.int8
    d_wblob = nc.dram_tensor("wblob", [WSH], I8, kind="ExternalInput")
    d_wscale = nc.dram_tensor("wscale", [SSH], F32, kind="ExternalInput")
    d_xidx = nc.dram_tensor("xidx", [16, NBT // 16], U16, kind="ExternalInput")
    d_hsel = nc.dram_tensor("hsel", [128, 1], U16, kind="ExternalInput")
    d_wlin = nc.dram_tensor("wlin", [128, 2], F32, kind="ExternalInput")
    d_blin = nc.dram_tensor("blin", [1, 1], F32, kind="ExternalInput")

    d_y = nc.dram_tensor("y", [1, BL], F32, kind="ExternalOutput")
    if debug:
        d_out1Tf = nc.dram_tensor("dbg_out1Tf", [128, NBT], BF16, kind="ExternalOutput")
        d_out1Tb = nc.dram_tensor("dbg_out1Tb", [128, NBT], BF16, kind="ExternalOutput")
        d_attT0 = nc.dram_tensor("dbg_attT0", [128, NBT], BF16, kind="ExternalOutput")
        d_attT1 = nc.dram_tensor("dbg_attT1", [128, NBT], BF16, kind="ExternalOutput")
        d_out2Tf = nc.dram_tensor("dbg_out2Tf", [128, NBT], BF16, kind="ExternalOutput")
        d_out2Tb = nc.dram_tensor("dbg_out2Tb", [128, NBT], BF16, kind="ExternalOutput")

    with tile.TileContext(nc) as tc:
        with tc.tile_pool(name="pers", bufs=1) as pers, \
             tc.tile_pool(name="work", bufs=3) as work, \
             tc.tile_pool(name="psg", bufs=3, space="PSUM") as psg, \
             tc.tile_pool(name="psb", bufs=3, space="PSUM") as psb, \
             tc.tile_pool(name="pss", bufs=2, space="PSUM") as pss, \
             tc.tile_pool(name="dram", bufs=1, space="DRAM") as dram:

            # ---------------- persistent SBUF ----------------
            embT = pers.tile([128, V], BF16, tag="embT")
            eT = pers.tile([128, NBT], BF16, tag="eT")      # rows 0..9 e, row 10 ones
            xg = pers.tile([128, G8 * NBT], BF16, tag="xg")
            outTf = pers.tile([128, NBT], BF16, tag="outTf")
            outTb = pers.tile([128, NBT], BF16, tag="outTb")
            out1 = pers.tile([128, BL * TC * 256], BF16, tag="out1")  # [t, d] per b
            Fw = [pers.tile([128, NBT], BF16, tag=f"F{tcx}", name=f"F{tcx}") for tcx in range(TC)]
            attT = [pers.tile([128, NBT], BF16, tag=f"attT{dc}", name=f"attT{dc}") for dc in range(2)]
            Dloc = pers.tile([128, TC * T], F32, tag="Dloc")  # reused as Drec
            hgath = pers.tile([128, 128], BF16, tag="hgath")
            hid = pers.tile([128, 16], BF16, tag="hid")
            ate = pers.tile([128, TC * BL], F32, tag="ate")
            at1 = pers.tile([128, TC * BL], BF16, tag="at1")
            dloc_s = pers.tile([128, TC], F32, tag="dlocs")
            drec_s = pers.tile([128, TC], F32, tag="drecs")
            a2sb = pers.tile([128, 2 * BL], F32, tag="a2sb")
            ysb = pers.tile([1, BL], F32, tag="ysb")

            w_ident = pers.tile([128, 128], BF16, tag="ident")
            invbc = pers.tile([128, T], F32, tag="invbc")
            wxg1 = pers.tile([E + 1, 4 * H], BF16, tag="wxg1")   # fwd
            wxg1b = pers.tile([E + 1, 4 * H], BF16, tag="wxg1b")  # bwd
            whh1 = [pers.tile([H, 4 * H], BF16, tag=f"whh1{d}", name=f"whh1{d}") for d in range(2)]
            whh2 = [pers.tile([H, 4 * H], BF16, tag=f"whh2{d}", name=f"whh2{d}") for d in range(2)]
            wxg2 = [[pers.tile([128, 4 * H], BF16, tag=f"wxg2{d}{k}", name=f"wxg2{d}{k}") for k in range(2)]
                    for d in range(2)]
            wxg2c = [pers.tile([1, 4 * H], BF16, tag=f"wxg2c{d}", name=f"wxg2c{d}") for d in range(2)]
            hselt = pers.tile([128, 1], U16, tag="hsel")
            xidxt = pers.tile([128, NBT // 16], U16, tag="xidx")
            wlin = pers.tile([128, 2], F32, tag="wlin")
            ones1 = pers.tile([1, T], BF16, tag="ones1")
            blin = pers.tile([1, 1], F32, tag="blin")

            # ---------------- DRAM bounce buffers ----------------
            db_in = dram.tile([T, T], F32, tag="dbin")
            db_out = dram.tile([T, T], F32, tag="dbout")
            hb_in = dram.tile([128, 16], BF16, tag="hbin")
            hb_out = dram.tile([NCORES * 128, 16], BF16, tag="hbout")
            sb_in = dram.tile([128, TC], F32, tag="sbin")
            sb_out = dram.tile([128, TC], F32, tag="sbout")
            hb2_in = dram.tile([128, 16], BF16, tag="hb2in")
            hb2_out = dram.tile([NCORES * 128, 16], BF16, tag="hb2out")
            sb2_in = dram.tile([128, TC], F32, tag="sb2in")
            sb2_out = dram.tile([128, TC], F32, tag="sb2out")
            wb_in = dram.tile([WSH], I8, tag="wbin")
            wb = dram.tile([NCORES * WSH], I8, tag="wblob")
            ws_in = dram.tile([SSH], F32, tag="wsin")
            ws = dram.tile([NCORES * SSH], F32, tag="wscale")

            # ---------------- gather replicated weights ----------------
            # collectives may not read IO tensors: bounce the shards first
            nc.sync.dma_start(wb_in[:], d_wblob.ap())
            nc.sync.dma_start(ws_in[:], d_wscale.ap())
            nc.gpsimd.collective_compute(
                "AllGather", ALU.bypass, replica_groups=[list(range(NCORES))],
                ins=[wb_in.opt()], outs=[wb.opt()])
            nc.gpsimd.collective_compute(
                "AllGather", ALU.bypass, replica_groups=[list(range(NCORES))],
                ins=[ws_in.opt()], outs=[ws.opt()])

            def wb_view(off, r, c):
                return wb[off: off + r * c].rearrange("(r c) -> r c", c=c)

            def load_q8(dst_ap, off, soff, r, c):
                """DMA int8 rows + per-row scale, dequantize into dst (bf16)."""
                st = work.tile([128, V], I8, tag="q8st")
                sc = work.tile([128, 1], F32, tag="q8sc")
                nc.sync.dma_start(st[0:r, 0:c], wb_view(off, r, c))
                nc.sync.dma_start(
                    sc[0:r, :],
                    ws[soff: soff + r].rearrange("(r c) -> r c", c=1))
                nc.vector.tensor_scalar_mul(dst_ap, st[0:r, 0:c], sc[0:r, :])

            # ---------------- load constants ----------------
            load_q8(wxg1[:], OFF_WXG1, SOFF_WXG1, E + 1, 4 * H)
            load_q8(wxg1b[:], OFF_WXG1 + 11 * 512, SOFF_WXG1 + 11, E + 1, 4 * H)
            for d in range(2):
                load_q8(whh1[d][:], OFF_WHH1 + d * 128 * 512,
                        SOFF_WHH1 + d * 128, H, 4 * H)
                load_q8(whh2[d][:], OFF_WHH2 + d * 128 * 512,
                        SOFF_WHH2 + d * 128, H, 4 * H)
                b8 = OFF_WXG2 + d * 257 * 512
                sb8 = SOFF_WXG2 + d * 257
                load_q8(wxg2[d][0][:], b8, sb8, 128, 4 * H)
                load_q8(wxg2[d][1][:], b8 + 128 * 512, sb8 + 128, 128, 4 * H)
                load_q8(wxg2c[d][:], b8 + 256 * 512, sb8 + 256, 1, 4 * H)
            nc.sync.dma_start(hselt[:], d_hsel.ap())
            for g in range(8):
                nc.sync.dma_start(xidxt[16 * g:16 * (g + 1), :], d_xidx.ap())
            nc.sync.dma_start(wlin[:], d_wlin.ap())
            nc.sync.dma_start(blin[:], d_blin.ap())

            # identity matrix: ones masked to the diagonal (iota = col - p)
            nc.vector.memset(w_ident[:], 1.0)
            nc.gpsimd.affine_select(
                w_ident[:], w_ident[:], pattern=[[1, 128]],
                compare_op=ALU.is_ge, fill=0.0, base=0, channel_multiplier=-1)
            nc.gpsimd.affine_select(
                w_ident[:], w_ident[:], pattern=[[-1, 128]],
                compare_op=ALU.is_ge, fill=0.0, base=0, channel_multiplier=1)

            # invbc[p, t] = 1 / max(t, 1)
            iv32 = work.tile([128, T], mybir.dt.int32, tag="iv32")
            nc.gpsimd.iota(iv32[:], pattern=[[1, T]], base=0, channel_multiplier=0)
            nc.vector.tensor_copy(invbc[:], iv32[:])
            nc.vector.tensor_scalar_max(invbc[:], invbc[:], 1.0)
            nc.vector.reciprocal(invbc[:], invbc[:])

            # ---------------- phase A: embedding gather + xg1 ----------------
            nc.vector.memset(embT[:], 0.0)
            load_q8(embT[0:E + 1, :], OFF_EMBT, SOFF_EMBT, E + 1, V)
            for g in range(1, 8):
                nc.sync.dma_start(embT[16 * g:16 * g + E + 1, :],
                                  embT[0:E + 1, :])
            GCH = 512  # gather chunk (ISA dst-elem-count limit)
            for k in range((NBT + GCH - 1) // GCH):
                ch = min(GCH, NBT - k * GCH)
                nc.gpsimd.indirect_copy(
                    eT[:, k * GCH:k * GCH + ch], embT[:],
                    xidxt[:, k * GCH // 16:(k * GCH + ch) // 16], True)
            nc.vector.memset(ones1[:], 1.0)

            def xg_proj(lhsT_of, nk, rhs_of, evac_rev):
                """xg[, chunk m] = sum_k lhsT_k.T @ rhs_k ; evac (reversed for bwd)."""
                for m in range(G8):          # chunk (2g+d)
                    g, d = divmod(m, 2)
                    for b in range(BL):
                        ps = psb.tile([128, PSW], F32, tag="big")
                        for k in range(nk):
                            nc.tensor.matmul(
                                ps[:, 0:T], lhsT_of(d, g, k), rhs_of(d, k, b),
                                start=(k == 0), stop=(k == nk - 1),
                            )
                        dst = xg[:, m * NBT + b * T: m * NBT + (b + 1) * T]
                        if d == 1 and evac_rev:
                            dst = dst[:, ::-1]
                        nc.vector.tensor_copy(dst, ps[:, 0:T])

            # layer-1 projection: K = 11 (E rows + ones)
            xg_proj(
                lhsT_of=lambda d, g, k: (wxg1 if d == 0 else wxg1b)[:, g * H:(g + 1) * H],
                nk=1,
                rhs_of=lambda d, k, b: eT[0:E + 1, b * T:(b + 1) * T],
                evac_rev=True,
            )

            # ---------------- recurrence (both layers) ----------------
            def recurrence(whh, oTf, oTb):
                oTf_r = oTf[:].rearrange("p (b t) -> p b t", b=BL)
                oTb_r = oTb[:].rearrange("p (b t) -> p b t", b=BL)
                xg_r = xg[:].rearrange("p (c b t) -> p c b t", c=G8, b=BL)
                c_prev = None
                for t in range(T):
                    ps = psg.tile([128, 64], F32, tag="g")
                    nc.tensor.matmul(ps[:, 0:64], w_ident[:], xg_r[:, :, :, t],
                                     start=True, stop=(t == 0))
                    if t > 0:
                        for d in range(2):
                            tau = (t - 1) if d == 0 else (T - t)
                            h_ap = (oTf_r if d == 0 else oTb_r)[:, :, tau]
                            for g in range(4):
                                nc.tensor.matmul(
                                    ps[:, (2 * g + d) * BL:(2 * g + d + 1) * BL],
                                    whh[d][:, g * H:(g + 1) * H], h_ap,
                                    start=False, stop=(d == 1 and g == 3),
                                )
                    sig = work.tile([128, 64], F32, tag="sig")
                    nc.scalar.activation(sig[:, 0:48], ps[:, 0:48], AF.Sigmoid)
                    nc.scalar.activation(sig[:, 48:64], ps[:, 48:64], AF.Tanh)
                    cn = work.tile([128, 16], F32, tag="c")
                    if t > 0:
                        m1 = work.tile([128, 16], F32, tag="m1")
                        m2 = work.tile([128, 16], F32, tag="m2")
                        nc.vector.tensor_tensor(m2[:], sig[:, 0:16], sig[:, 48:64], ALU.mult)
                        nc.vector.tensor_tensor(m1[:], sig[:, 16:32], c_prev[:], ALU.mult)
                        nc.vector.tensor_tensor(cn[:], m1[:], m2[:], ALU.add)
                    else:
                        nc.vector.tensor_tensor(cn[:], sig[:, 0:16], sig[:, 48:64], ALU.mult)
                    th = work.tile([128, 16], F32, tag="th")
                    nc.scalar.activation(th[:], cn[:], AF.Tanh)
                    nc.vector.tensor_tensor(oTf_r[:, :, t], sig[:, 32:40], th[:, 0:8], ALU.mult)
                    nc.vector.tensor_tensor(oTb_r[:, :, T - 1 - t], sig[:, 40:48], th[:, 8:16], ALU.mult)
                    c_prev = cn

            recurrence(whh1, outTf, outTb)

            # ---------------- phase C: prefix + full attention (layer 1) -------
            outT = [outTf, outTb]
            # S/F: F[tc][:, b*T + i] = exp(sum_d out1[t',d] * out1[i,d] * inv(i))
            for b in range(BL):
                # per-batch scaled copies (rhs of the S matmul)
                outs_b = [work.tile([128, T], BF16, tag=f"outs{dc}", name=f"outs{dc}") for dc in range(2)]
                for dc in range(2):
                    nc.vector.tensor_tensor(
                        outs_b[dc][:], outT[dc][:, b * T:(b + 1) * T],
                        invbc[:], ALU.mult)
                for tcx in range(TC):
                    tch = min(128, T - tcx * 128)
                    ps = psb.tile([128, PSW], F32, tag="big")
                    for dc in range(2):
                        nc.tensor.matmul(
                            ps[0:tch, 0:T],
                            outT[dc][:, b * T + tcx * 128: b * T + tcx * 128 + tch],
                            outs_b[dc][:],
                            start=(dc == 0), stop=(dc == 1),
                        )
                    nc.scalar.activation(Fw[tcx][0:tch, b * T:(b + 1) * T],
                                         ps[0:tch, 0:T], AF.Exp)

            # local denominator sums over b: Dloc[:, tc*T + i]
            for tcx in range(TC):
                tch = min(128, T - tcx * 128)
                fr = Fw[tcx][0:tch].rearrange("p (b t) -> p t b", b=BL)
                nc.vector.tensor_reduce(
                    Dloc[0:tch, tcx * T:(tcx + 1) * T], fr,
                    axis=mybir.AxisListType.X, op=ALU.add)

            # h1 finals -> hb_in: cols 0:8 fwd (t=T-1), 8:16 bwd (t=0)
            oTf_r = outTf[:].rearrange("p (b t) -> p b t", b=BL)
            oTb_r = outTb[:].rearrange("p (b t) -> p b t", b=BL)
            hfin = work.tile([128, 16], BF16, tag="hfin")
            nc.vector.tensor_copy(hfin[:, 0:8], oTf_r[:, :, T - 1])
            nc.vector.tensor_copy(hfin[:, 8:16], oTb_r[:, :, 0])
            nc.sync.dma_start(hb_in[:], hfin[:])
            for tcx in range(TC):
                tch = min(128, T - tcx * 128)
                nc.sync.dma_start(db_in[tcx * 128:tcx * 128 + tch, :],
                                  Dloc[0:tch, tcx * T:(tcx + 1) * T])
            nc.gpsimd.collective_compute(
                "AllReduce", ALU.add, replica_groups=[list(range(NCORES))],
                ins=[db_in.opt()], outs=[db_out.opt()])
            nc.gpsimd.collective_compute(
                "AllGather", ALU.bypass, replica_groups=[list(range(NCORES))],
                ins=[hb_in.opt()], outs=[hb_out.opt()])
            for tcx in range(TC):
                tch = min(128, T - tcx * 128)
                nc.sync.dma_start(Dloc[0:tch, tcx * T:(tcx + 1) * T],
                                  db_out[tcx * 128:tcx * 128 + tch, :])
            nc.sync.dma_start(
                hgath[:], hb_out[:].rearrange("(c p) j -> p c j", p=128))

            # transpose out1T -> out1 [t, d] (per b, tc, dc), bf16
            for b in range(BL):
                for tcx in range(TC):
                    tch = min(128, T - tcx * 128)
                    for dc in range(2):
                        pt = psb.tile([128, PSW], BF16, tag="big")
                        nc.tensor.transpose(
                            pt[0:tch, 0:128],
                            outT[dc][:, b * T + tcx * 128: b * T + tcx * 128 + tch],
                            w_ident[:])
                        nc.vector.tensor_copy(
                            out1[0:tch, (b * TC + tcx) * 256 + dc * 128:
                                 (b * TC + tcx) * 256 + dc * 128 + 128],
                            pt[0:tch, 0:128])

            # reciprocal + strict lower-triangular mask on the denominators
            for tcx in range(TC):
                tch = min(128, T - tcx * 128)
                nc.vector.reciprocal(Dloc[0:tch, tcx * T:(tcx + 1) * T],
                                     Dloc[0:tch, tcx * T:(tcx + 1) * T])
                nc.gpsimd.affine_select(
                    Dloc[0:tch, tcx * T:(tcx + 1) * T],
                    Dloc[0:tch, tcx * T:(tcx + 1) * T],
                    pattern=[[1, T]], compare_op=ALU.is_gt, fill=0.0,
                    base=-tcx * 128, channel_multiplier=-1)

            # W~ = F * 1/D (masked), in place
            for tcx in range(TC):
                tch = min(128, T - tcx * 128)
                for b in range(BL):
                    nc.vector.tensor_tensor(
                        Fw[tcx][0:tch, b * T:(b + 1) * T],
                        Fw[tcx][0:tch, b * T:(b + 1) * T],
                        Dloc[0:tch, tcx * T:(tcx + 1) * T], ALU.mult)

            # att^T[dc][:, b*T + i] = sum_t out1[t, d] W~[t, i]
            for b in range(BL):
                for dc in range(2):
                    ps = psb.tile([128, PSW], F32, tag="big")
                    for tcx in range(TC):
                        tch = min(128, T - tcx * 128)
                        nc.tensor.matmul(
                            ps[:, 0:T],
                            out1[0:tch, (b * TC + tcx) * 256 + dc * 128:
                                 (b * TC + tcx) * 256 + dc * 128 + 128],
                            Fw[tcx][0:tch, b * T:(b + 1) * T],
                            start=(tcx == 0), stop=(tcx == TC - 1),
                        )
                    nc.vector.tensor_copy(attT[dc][:, b * T:(b + 1) * T], ps[:, 0:T])

            # ---- full attention #1 (scores vs torch-reshaped h_n) ----
            def full_attention(oT_pair, out_sb, attdst):
                """scores from oT_pair lhsT + hid rhs; writes at into `at1`;
                returns after computing a2 columns into attdst (list per dc)."""
                nc.gpsimd.indirect_copy(hid[:], hgath[:], hselt[:], True)
                sc = pss.tile([128, TC * BL], F32, tag="small")
                if T % 128 != 0:
                    nc.vector.memset(sc[:], 0.0)
                for b in range(BL):
                    for tcx in range(TC):
                        tch = min(128, T - tcx * 128)
                        for dc in range(2):
                            nc.tensor.matmul(
                                sc[0:tch, tcx * BL + b: tcx * BL + b + 1],
                                oT_pair[dc][:, b * T + tcx * 128: b * T + tcx * 128 + tch],
                                hid[:, 2 * b + dc: 2 * b + dc + 1],
                                start=(dc == 0), stop=(dc == 1),
                            )
                nc.scalar.activation(ate[:], sc[:], AF.Exp, scale=1.0 / T)
                ar = ate[:].rearrange("p (t b) -> p t b", b=BL)
                nc.vector.tensor_reduce(dloc_s[:], ar, axis=mybir.AxisListType.X,
                                        op=ALU.add)
                return sc

            sc1 = full_attention([outTf, outTb], out1, attT)
            nc.sync.dma_start(sb_in[:], dloc_s[:])
            nc.gpsimd.collective_compute(
                "AllReduce", ALU.add, replica_groups=[list(range(NCORES))],
                ins=[sb_in.opt()], outs=[sb_out.opt()])
            nc.sync.dma_start(drec_s[:], sb_out[:])
            nc.vector.reciprocal(drec_s[:], drec_s[:])

            def finish_attention(attdst, col):
                """at = ate/d ; a2^T[dc] = sum_t out1[t,d] at[t] -> attdst[dc][:, col+b*T]"""
                for b in range(BL):
                    nc.vector.tensor_tensor(
                        at1[:].rearrange("p (t b) -> p t b", b=BL)[:, :, b],
                        ate[:].rearrange("p (t b) -> p t b", b=BL)[:, :, b],
                        drec_s[:], ALU.mult)
                for b in range(BL):
                    for dc in range(2):
                        pa = pss.tile([128, TC * BL], F32, tag="small")
                        for tcx in range(TC):
                            tch = min(128, T - tcx * 128)
                            nc.tensor.matmul(
                                pa[0:128, 0:1],
                                out1[0:tch, (b * TC + tcx) * 256 + dc * 128:
                                     (b * TC + tcx) * 256 + dc * 128 + 128],
                                at1[0:tch, tcx * BL + b: tcx * BL + b + 1],
                                start=(tcx == 0), stop=(tcx == TC - 1),
                            )
                        if attdst is not None:
                            nc.vector.tensor_copy(
                                attdst[dc][:, b * T + col: b * T + col + 1],
                                pa[:, 0:1])
                        else:
                            nc.vector.tensor_copy(
                                a2sb[:, b * 2 + dc: b * 2 + dc + 1], pa[:, 0:1])

            finish_attention(attT, T - 1)

            # ---------------- phase D: xg2 projection ----------------
            rhs2 = [attT[0], attT[1]]
            for m in range(G8):
                g, d = divmod(m, 2)
                for b in range(BL):
                    ps = psb.tile([128, PSW], F32, tag="big")
                    for k in range(2):
                        nc.tensor.matmul(
                            ps[:, 0:T], wxg2[d][k][:, g * H:(g + 1) * H],
                            rhs2[k][:, b * T:(b + 1) * T],
                            start=(k == 0), stop=False)
                    nc.tensor.matmul(
                        ps[:, 0:T], wxg2c[d][:, g * H:(g + 1) * H],
                        ones1[:],
                        start=False, stop=True)
                    dst = xg[:, m * NBT + b * T: m * NBT + (b + 1) * T]
                    if d == 1:
                        dst = dst[:, ::-1]
                    nc.vector.tensor_copy(dst, ps[:, 0:T])

            # ---------------- phase E: layer-2 recurrence ----------------
            if debug:
                nc.sync.dma_start(d_out1Tf.ap(), outTf[:])
                nc.sync.dma_start(d_out1Tb.ap(), outTb[:])
                nc.sync.dma_start(d_attT0.ap(), attT[0][:])
                nc.sync.dma_start(d_attT1.ap(), attT[1][:])
            recurrence(whh2, outTf, outTb)
            if debug:
                nc.sync.dma_start(d_out2Tf.ap(), outTf[:])
                nc.sync.dma_start(d_out2Tb.ap(), outTb[:])

            # ---------------- phase F: final full attention + linear ----------
            # h2 finals gather
            hfin2 = work.tile([128, 16], BF16, tag="hfin")
            nc.vector.tensor_copy(hfin2[:, 0:8], oTf_r[:, :, T - 1])
            nc.vector.tensor_copy(hfin2[:, 8:16], oTb_r[:, :, 0])
            nc.sync.dma_start(hb2_in[:], hfin2[:])
            nc.gpsimd.collective_compute(
                "AllGather", ALU.bypass, replica_groups=[list(range(NCORES))],
                ins=[hb2_in.opt()], outs=[hb2_out.opt()])
            nc.sync.dma_start(
                hgath[:], hb2_out[:].rearrange("(c p) j -> p c j", p=128))

            # transpose out2T -> out1 buffer ([t, d] layout)
            for b in range(BL):
                for tcx in range(TC):
                    tch = min(128, T - tcx * 128)
                    for dc in range(2):
                        pt = psb.tile([128, PSW], BF16, tag="big")
                        nc.tensor.transpose(
                            pt[0:tch, 0:128],
                            outT[dc][:, b * T + tcx * 128: b * T + tcx * 128 + tch],
                            w_ident[:])
                        nc.vector.tensor_copy(
                            out1[0:tch, (b * TC + tcx) * 256 + dc * 128:
                                 (b * TC + tcx) * 256 + dc * 128 + 128],
                            pt[0:tch, 0:128])

            sc2 = full_attention([outTf, outTb], out1, None)
            nc.sync.dma_start(sb2_in[:], dloc_s[:])
            nc.gpsimd.collective_compute(
                "AllReduce", ALU.add, replica_groups=[list(range(NCORES))],
                ins=[sb2_in.opt()], outs=[sb2_out.opt()])
            nc.sync.dma_start(drec_s[:], sb2_out[:])
            nc.vector.reciprocal(drec_s[:], drec_s[:])
            finish_attention(None, 0)

            # y = sigmoid(a2 @ w + b)
            py = pss.tile([128, TC * BL], F32, tag="small")
            a2r = a2sb[:].rearrange("p (b k) -> p b k", k=2)
            for dc in range(2):
                nc.tensor.matmul(py[0:1, 0:BL], wlin[:, dc:dc + 1], a2r[:, :, dc],
                                 start=(dc == 0), stop=(dc == 1))
            nc.scalar.activation(ysb[:], py[0:1, 0:BL], AF.Sigmoid, bias=blin[:])
            nc.sync.dma_start(d_y.ap(), ysb[:])

    nc.compile()
    return nc


# ======================= host-side wrapper =======================

def _q8_rows(w):
    """[R, C] f32 -> (int8 [R, C], f32 scale [R]) symmetric per-row quant."""
    w = np.asarray(w, np.float32)
    s = np.abs(w).max(axis=1) / 127.0
    s = np.where(s == 0, 1.0, s).astype(np.float32)
    q = np.round(w / s[:, None]).clip(-127, 127).astype(np.int8)
    return q, s


GATE_PERM = [0, 1, 3, 2]  # torch (i,f,g,o) chunks -> ours (i,f,o,g)


def _reorder_gates(w):
    """w [4H, ...] in torch gate order -> [4H, ...] in (i,f,o,g) order."""
    chunks = [w[g * H:(g + 1) * H] for g in GATE_PERM]
    return np.concatenate(chunks, axis=0)


def _pack_xgw(Wih, bih, bhh):
    """-> [K+1, 4H] rows: Wih^T then combined bias row (gate-reordered)."""
    Wr = _reorder_gates(np.asarray(Wih))          # [4H, K]
    br = _reorder_gates((np.asarray(bih) + np.asarray(bhh))[:, None])[:, 0]  # [4H]
    return np.concatenate([Wr.T, br[None, :]], axis=0)  # [K+1, 4H]


def _pack_whh(Whh):
    return _reorder_gates(np.asarray(Whh)).T      # [H, 4H]


def _wrap16(flat):
    """flat [N] -> [16, N//16] wrapped (s p) per 16-group (replicated on device)."""
    return np.ascontiguousarray(flat.reshape(-1, 16).T)  # [16, s]


def prepare_inputs(T, x, emb, l1_Wih_f, l1_Whh_f, l1_bih_f, l1_bhh_f,
                   l1_Wih_b, l1_Whh_b, l1_bih_b, l1_bhh_b,
                   l2_Wih_f, l2_Whh_f, l2_bih_f, l2_bhh_f,
                   l2_Wih_b, l2_Whh_b, l2_bih_b, l2_bhh_b, lin_W, lin_b):
    """Build per-core in_maps."""
    x = np.asarray(x).astype(np.int64)
    embT = np.concatenate(
        [np.asarray(emb, np.float32).T, np.ones((1, V), np.float32)], axis=0)
    pieces = [
        _pack_xgw(l1_Wih_f, l1_bih_f, l1_bhh_f),
        _pack_xgw(l1_Wih_b, l1_bih_b, l1_bhh_b),
        _pack_whh(l1_Whh_f),
        _pack_whh(l1_Whh_b),
        _pack_xgw(l2_Wih_f, l2_bih_f, l2_bhh_f),
        _pack_xgw(l2_Wih_b, l2_bih_b, l2_bhh_b),
        _pack_whh(l2_Whh_f),
        _pack_whh(l2_Whh_b),
        embT,
    ]
    qs = [_q8_rows(p) for p in pieces]
    blob = np.concatenate([q.ravel() for q, _ in qs])
    scales = np.concatenate([s for _, s in qs])
    assert blob.shape[0] == WBLOB_LEN and scales.shape[0] == SLEN
    blob = np.concatenate(
        [blob, np.zeros(NCORES * WSH - WBLOB_LEN, dtype=np.int8)])
    scales = np.concatenate(
        [scales, np.zeros(NCORES * SSH - SLEN, dtype=np.float32)])
    wshards = blob.reshape(NCORES, WSH)
    sshards = scales.reshape(NCORES, SSH)
    shared = {
        "wlin": np.asarray(lin_W, dtype=np.float32).reshape(256)
                  .reshape(2, 128).T.copy(),
        "blin": np.asarray(lin_b, dtype=np.float32).reshape(1, 1),
    }
    in_maps = []
    for c in range(NCORES):
        xl = x[c * BL:(c + 1) * BL, :]            # [BL, T]
        xflat = xl.reshape(-1).astype(np.uint16)  # b-major
        # hidden-selection gather indices for this core (torch h_n reshape)
        L = np.zeros(16, dtype=np.uint16)
        for bl in range(BL):
            bglob = c * BL + bl
            for k in range(2):
                if bglob < B // 2:
                    gidx = 2 * bglob + k
                    col = (gidx // BL) * 16 + (gidx % BL)
                else:
                    gidx = 2 * bglob - B + k
                    col = (gidx // BL) * 16 + 8 + (gidx % BL)
                L[2 * bl + k] = col
        hsel = np.zeros((128, 1), dtype=np.uint16)
        for g in range(8):
            hsel[16 * g:16 * (g + 1), 0] = L
        m = dict(shared)
        m["wblob"] = wshards[c]
        m["wscale"] = sshards[c]
        m["xidx"] = _wrap16(xflat)
        m["hsel"] = hsel
        in_maps.append(m)
    return in_maps


_CACHE = {}
_RUN_CACHE = {}


def _make_runner(nc, n_cores=NCORES):
    """Build a cached jitted PJRT runner (mirrors bass2jax.run_bass_via_pjrt)."""
    import jax
    from jax.experimental.shard_map import shard_map
    from jax.sharding import Mesh, PartitionSpec
    from concourse import bass2jax

    bass2jax.install_neuronx_cc_hook()
    partition_name = (nc.partition_id_tensor.name
                      if nc.partition_id_tensor else None)
    in_names, out_names, out_avals, zero_shapes = [], [], [], []
    for alloc in nc.m.functions[0].allocations:
        if not isinstance(alloc, mybir.MemoryLocationSet):
            continue
        name = alloc.memorylocations[0].name
        if alloc.kind == "ExternalInput":
            if name != partition_name:
                in_names.append(name)
        elif alloc.kind == "ExternalOutput":
            shape = tuple(alloc.tensor_shape)
            dtype = mybir.dt.np(alloc.dtype)
            out_names.append(name)
            out_avals.append(jax.core.ShapedArray(shape, dtype))
            zero_shapes.append((shape, dtype))
    n_params = len(in_names)
    n_outs = len(out_avals)
    all_names = in_names + out_names
    if partition_name is not None:
        all_names.append(partition_name)
    donate = tuple(range(n_params, n_params + n_outs))

    def _body(*args):
        operands = list(args)
        if partition_name is not None:
            operands.append(bass2jax.partition_id_tensor())
        outs = bass2jax._bass_exec_p.bind(
            *operands, out_avals=tuple(out_avals), in_names=tuple(all_names),
            out_names=tuple(out_names), lowering_input_output_aliases=(),
            sim_require_finite=False, sim_require_nnan=False, nc=nc)
        return tuple(outs)

    devices = jax.devices()[:n_cores]
    mesh = Mesh(np.asarray(devices), ("core",))
    in_specs = (PartitionSpec("core"),) * (n_params + n_outs)
    out_specs = (PartitionSpec("core"),) * n_outs
    sharded = jax.jit(
        shard_map(_body, mesh=mesh, in_specs=in_specs, out_specs=out_specs,
                  check_rep=False),
        donate_argnums=donate, keep_unused=True)

    def runner(in_maps):
        per_core = [[np.asarray(m[n]) for n in in_names] for m in in_maps]
        concat_in = [np.concatenate([per_core[c][i] for c in range(n_cores)],
                                    axis=0) for i in range(n_params)]
        concat_zeros = [np.zeros((n_cores * s[0], *s[1:]), d)
                        for (s, d) in zero_shapes]
        out_arrs = sharded(*concat_in, *concat_zeros)
        out_arrs = [np.asarray(a) for a in out_arrs]
        return [
            {name: out_arrs[i].reshape(n_cores, *out_avals[i].shape)[c]
             for i, name in enumerate(out_names)}
            for c in range(n_cores)]

    return runner


def get_runner(T, debug=False):
    key = (T, debug)
    if key not in _RUN_CACHE:
        _RUN_CACHE[key] = _make_runner(_get_program(T, debug))
    return _RUN_CACHE[key]


def _get_program(T, debug):
    key = (T, debug)
    if key not in _CACHE:
        _CACHE[key] = build_program(T, debug)
    return _CACHE[key]


def run(T, inputs, debug=False, trace=False):
    nc = _get_program(T, debug)
    in_maps = prepare_inputs(T, **inputs)
    res = bass_utils.run_bass_kernel_spmd(
        nc, in_maps, core_ids=list(range(NCORES)), trace=trace)
    y = np.concatenate([res.results[c]["y"].reshape(BL) for c in range(NCORES)])
    return y.reshape(B, 1).astype(np.float32), res


def kernel(**inputs) -> np.ndarray:
    T = np.asarray(inputs["x"]).shape[1]
    try:
        y, _ = run(T, inputs, debug=False, trace=False)
    except Exception:
        # transient NRT/device errors (e.g. NRT_EXEC_UNIT_UNRECOVERABLE)
        # usually clear on retry
        import time
        time.sleep(15)
        y, _ = run(T, inputs, debug=False, trace=False)
    return y



# revision 24
# speedup vs baseline: 1.1110x; 1.1110x over previous
"""Trainium2 Bass kernel for a 2-layer BiLSTM with legacy softmax-over-batch
attention (nn_BILSTM_withAttention2layer).

Sharding: data-parallel over batch B=64 across 8 NeuronCores (8 batches per
core). All weights replicated on device. The legacy softmax over the *batch*
axis in both attention blocks is handled with on-device collectives:
  - AllReduce(add) of per-core exp-sums for the prefix-attention denominators
  - AllReduce(add) of per-core exp-sums for the two full-attention softmaxes
  - AllGather of the per-direction final hidden states (the torch-faithful
    h_n.view(B, 2H) mixes batches, so every core needs other cores' finals)

Host->device I/O is latency/bandwidth-dominated (axon tunnel), so the wrapper
ships the minimum bytes per call:
  - all weights + embedding table in ONE int8 blob (symmetric per-row quant,
    f32 scales in a parallel blob), sharded 1/8 per core and AllGathered +
    dequantized on device
  - per-core token indices only ([16, BL*T/16] u16, replicated to the 8
    partition groups on device)
  - ident / inv-prefix-count / ones constants generated on device
    (iota + affine_select) instead of shipped

Layouts (per core, bl = 8 local batches):
  - time-major "T" tensors [128, bl*T] with column  b*T + t
  - LSTM state/gates kept as [H=128 partitions, (gate,dir,b) free]
  - gates PSUM bank [128, 64]: col (2g+d)*8 + b, gate order (i, f, o, g)
  - xg (input projections) precomputed as bf16 [128, 8*bl*T], chunk (2g+d);
    backward-direction chunks stored time-reversed so the recurrence reads
    a uniform forward index.
"""

import os
import dataclasses
import numpy as np

import concourse.bass as bass
import concourse.mybir as mybir
import concourse.tile as tile
from concourse import bacc
from concourse import bass_utils

F32 = mybir.dt.float32
BF16 = mybir.dt.bfloat16
U16 = mybir.dt.uint16
AF = mybir.ActivationFunctionType
ALU = mybir.AluOpType

H = 128
B = 64
NCORES = 8
BL = B // NCORES  # 8
E = 10
V = 1002

# Packed replicated-weight blob, sharded 1/NCORES per core and AllGathered
# on device. Big matrices are int6 (4 vals per 3 bytes, symmetric per-row
# quant); small/sensitive pieces (layer-1 input proj, biases, embedding)
# stay int8. f32 per-row scales live in a parallel blob. Offsets in bytes:
PB = 384                                   # packed bytes per 512-col int6 row
OFF_WXG1 = 0                               # 2 x [E+1, 4H] int8
OFF_WHH1 = OFF_WXG1 + 2 * 11 * 512         # 2 x [H, PB] int6
OFF_WXG2W = OFF_WHH1 + 2 * 128 * PB        # 4 x [H, PB] int6 (d, half)
OFF_WXG2B = OFF_WXG2W + 4 * 128 * PB       # 2 x [1, 4H] int8 (bias rows)
OFF_WHH2 = OFF_WXG2B + 2 * 512             # 2 x [H, PB] int6
OFF_EMBT = OFF_WHH2 + 2 * 128 * PB         # [E+1, V] int8
WBLOB_LEN = OFF_EMBT + 11 * V              # 416526
WSH = (WBLOB_LEN + NCORES - 1) // NCORES   # per-core shard (pad to 8*WSH)
SOFF_WXG1 = 0
SOFF_WHH1 = SOFF_WXG1 + 2 * 11
SOFF_WXG2W = SOFF_WHH1 + 2 * 128
SOFF_WXG2B = SOFF_WXG2W + 4 * 128
SOFF_WHH2 = SOFF_WXG2B + 2
SOFF_EMBT = SOFF_WHH2 + 2 * 128
SLEN = SOFF_EMBT + 11                      # 1059 scale rows
SSH = (SLEN + NCORES - 1) // NCORES


def _bcast_b(ap2d, nb):
    """[128, N] -> [128, nb, N] with the batch dim broadcast (step 0)."""
    (ps, pc), (fs, fc) = ap2d.ap
    return dataclasses.replace(
        ap2d, ap=[[ps, pc], [0, nb], [fs, fc]]
    )


def build_program(T=512, debug=False):
    nc = bacc.Bacc(
        "TRN2", target_bir_lowering=False, debug=False,
        enable_asserts=False, num_devices=NCORES,
    )
    NBT = BL * T            # flattened (b, t) columns
    PSW = max(T, 128)       # psum big-tile width
    TC = (T + 127) // 128   # t-chunks
    G8 = 8                  # gate-dir chunks (i,f,o,g) x (fwd,bwd)

    # ---------------- DRAM I/O ----------------
    I8 = mybir.dt.int8
    d_wblob = nc.dram_tensor("wblob", [WSH], I8, kind="ExternalInput")
    d_wscale = nc.dram_tensor("wscale", [SSH], F32, kind="ExternalInput")
    d_xidx = nc.dram_tensor("xidx", [16, NBT // 16], U16, kind="ExternalInput")
    d_hsel = nc.dram_tensor("hsel", [128, 1], U16, kind="ExternalInput")
    d_wlin = nc.dram_tensor("wlin", [128, 2], F32, kind="ExternalInput")
    d_blin = nc.dram_tensor("blin", [1, 1], F32, kind="ExternalInput")

    d_y = nc.dram_tensor("y", [1, BL], F32, kind="ExternalOutput")
    if debug:
        d_out1Tf = nc.dram_tensor("dbg_out1Tf", [128, NBT], BF16, kind="ExternalOutput")
        d_out1Tb = nc.dram_tensor("dbg_out1Tb", [128, NBT], BF16, kind="ExternalOutput")
        d_attT0 = nc.dram_tensor("dbg_attT0", [128, NBT], BF16, kind="ExternalOutput")
        d_attT1 = nc.dram_tensor("dbg_attT1", [128, NBT], BF16, kind="ExternalOutput")
        d_out2Tf = nc.dram_tensor("dbg_out2Tf", [128, NBT], BF16, kind="ExternalOutput")
        d_out2Tb = nc.dram_tensor("dbg_out2Tb", [128, NBT], BF16, kind="ExternalOutput")

    with tile.TileContext(nc) as tc:
        with tc.tile_pool(name="pers", bufs=1) as pers, \
             tc.tile_pool(name="work", bufs=3) as work, \
             tc.tile_pool(name="psg", bufs=3, space="PSUM") as psg, \
             tc.tile_pool(name="psb", bufs=3, space="PSUM") as psb, \
             tc.tile_pool(name="pss", bufs=2, space="PSUM") as pss, \
             tc.tile_pool(name="dram", bufs=1, space="DRAM") as dram:

            # ---------------- persistent SBUF ----------------
            embT = pers.tile([128, V], BF16, tag="embT")
            eT = pers.tile([128, NBT], BF16, tag="eT")      # rows 0..9 e, row 10 ones
            xg = pers.tile([128, G8 * NBT], BF16, tag="xg")
            outTf = pers.tile([128, NBT], BF16, tag="outTf")
            outTb = pers.tile([128, NBT], BF16, tag="outTb")
            out1 = pers.tile([128, BL * TC * 256], BF16, tag="out1")  # [t, d] per b
            Fw = [pers.tile([128, NBT], BF16, tag=f"F{tcx}", name=f"F{tcx}") for tcx in range(TC)]
            attT = [pers.tile([128, NBT], BF16, tag=f"attT{dc}", name=f"attT{dc}") for dc in range(2)]
            Dloc = pers.tile([128, TC * T], F32, tag="Dloc")  # reused as Drec
            hgath = pers.tile([128, 128], BF16, tag="hgath")
            hid = pers.tile([128, 16], BF16, tag="hid")
            ate = pers.tile([128, TC * BL], F32, tag="ate")
            at1 = pers.tile([128, TC * BL], BF16, tag="at1")
            dloc_s = pers.tile([128, TC], F32, tag="dlocs")
            drec_s = pers.tile([128, TC], F32, tag="drecs")
            a2sb = pers.tile([128, 2 * BL], F32, tag="a2sb")
            ysb = pers.tile([1, BL], F32, tag="ysb")

            w_ident = pers.tile([128, 128], BF16, tag="ident")
            invbc = pers.tile([128, T], F32, tag="invbc")
            wxg1 = pers.tile([E + 1, 4 * H], BF16, tag="wxg1")   # fwd
            wxg1b = pers.tile([E + 1, 4 * H], BF16, tag="wxg1b")  # bwd
            whh1 = [pers.tile([H, 4 * H], BF16, tag=f"whh1{d}", name=f"whh1{d}") for d in range(2)]
            whh2 = [pers.tile([H, 4 * H], BF16, tag=f"whh2{d}", name=f"whh2{d}") for d in range(2)]
            wxg2 = [[pers.tile([128, 4 * H], BF16, tag=f"wxg2{d}{k}", name=f"wxg2{d}{k}") for k in range(2)]
                    for d in range(2)]
            wxg2c = [pers.tile([1, 4 * H], BF16, tag=f"wxg2c{d}", name=f"wxg2c{d}") for d in range(2)]
            hselt = pers.tile([128, 1], U16, tag="hsel")
            xidxt = pers.tile([128, NBT // 16], U16, tag="xidx")
            wlin = pers.tile([128, 2], F32, tag="wlin")
            ones1 = pers.tile([1, T], BF16, tag="ones1")
            blin = pers.tile([1, 1], F32, tag="blin")

            # ---------------- DRAM bounce buffers ----------------
            db_in = dram.tile([T, T], F32, tag="dbin")
            db_out = dram.tile([T, T], F32, tag="dbout")
            hb_in = dram.tile([128, 16], BF16, tag="hbin")
            hb_out = dram.tile([NCORES * 128, 16], BF16, tag="hbout")
            sb_in = dram.tile([128, TC], F32, tag="sbin")
            sb_out = dram.tile([128, TC], F32, tag="sbout")
            hb2_in = dram.tile([128, 16], BF16, tag="hb2in")
            hb2_out = dram.tile([NCORES * 128, 16], BF16, tag="hb2out")
            sb2_in = dram.tile([128, TC], F32, tag="sb2in")
            sb2_out = dram.tile([128, TC], F32, tag="sb2out")
            wb_in = dram.tile([WSH], I8, tag="wbin")
            wb = dram.tile([NCORES * WSH], I8, tag="wblob")
            ws_in = dram.tile([SSH], F32, tag="wsin")
            ws = dram.tile([NCORES * SSH], F32, tag="wscale")

            # ---------------- gather replicated weights ----------------
            # collectives may not read IO tensors: bounce the shards first
            nc.sync.dma_start(wb_in[:], d_wblob.ap())
            nc.sync.dma_start(ws_in[:], d_wscale.ap())
            nc.gpsimd.collective_compute(
                "AllGather", ALU.bypass, replica_groups=[list(range(NCORES))],
                ins=[wb_in.opt()], outs=[wb.opt()])
            nc.gpsimd.collective_compute(
                "AllGather", ALU.bypass, replica_groups=[list(range(NCORES))],
                ins=[ws_in.opt()], outs=[ws.opt()])

            def wb_view(off, r, c):
                return wb[off: off + r * c].rearrange("(r c) -> r c", c=c)

            U8 = mybir.dt.uint8

            def scale_view(soff, r):
                return ws[soff: soff + r].rearrange("(r c) -> r c", c=1)

            def load_q8(dst_ap, off, soff, r, c):
                """DMA int8 rows + per-row scale, dequantize into dst (bf16)."""
                st = work.tile([128, V], I8, tag="q8st")
                sc = work.tile([128, 1], F32, tag="q8sc")
                nc.sync.dma_start(st[0:r, 0:c], wb_view(off, r, c))
                nc.sync.dma_start(sc[0:r, :], scale_view(soff, r))
                nc.vector.tensor_scalar_mul(dst_ap, st[0:r, 0:c], sc[0:r, :])

            def load_q6(dst_ap, off, soff, r):
                """int6-packed rows (4 vals / 3 bytes, +32 bias) -> bf16."""
                st = work.tile([128, PB], U8, tag="q6st")
                sc = work.tile([128, 1], F32, tag="q6sc")
                ta = work.tile([128, 128], U8, tag="q6a")
                tb = work.tile([128, 128], U8, tag="q6b")
                nc.sync.dma_start(
                    st[0:r, :],
                    wb[off: off + r * PB].bitcast(U8)
                      .rearrange("(r c) -> r c", c=PB))
                nc.sync.dma_start(sc[0:r, :], scale_view(soff, r))
                b0 = st[0:r, 0:PB:3]
                b1 = st[0:r, 1:PB:3]
                b2 = st[0:r, 2:PB:3]
                A, B = ta[0:r, :], tb[0:r, :]
                s_ap = sc[0:r, :]

                def emit(v_ap, lane):
                    nc.vector.tensor_scalar(
                        dst_ap[0:r, lane:512:4], v_ap, 32.0, s_ap,
                        ALU.subtract, ALU.mult)

                nc.vector.tensor_scalar(A, b0, 63, None, ALU.bitwise_and)
                emit(A, 0)
                nc.vector.tensor_scalar(A, b0, 6, None, ALU.logical_shift_right)
                nc.vector.tensor_scalar(B, b1, 15, 2, ALU.bitwise_and,
                                        ALU.logical_shift_left)
                nc.vector.tensor_tensor(A, A, B, ALU.bitwise_or)
                emit(A, 1)
                nc.vector.tensor_scalar(A, b1, 4, 15, ALU.logical_shift_right,
                                        ALU.bitwise_and)
                nc.vector.tensor_scalar(B, b2, 3, 4, ALU.bitwise_and,
                                        ALU.logical_shift_left)
                nc.vector.tensor_tensor(A, A, B, ALU.bitwise_or)
                emit(A, 2)
                nc.vector.tensor_scalar(A, b2, 2, None, ALU.logical_shift_right)
                emit(A, 3)

            # ---------------- load constants ----------------
            load_q8(wxg1[:], OFF_WXG1, SOFF_WXG1, E + 1, 4 * H)
            load_q8(wxg1b[:], OFF_WXG1 + 11 * 512, SOFF_WXG1 + 11, E + 1, 4 * H)
            for d in range(2):
                load_q6(whh1[d][:], OFF_WHH1 + d * 128 * PB,
                        SOFF_WHH1 + d * 128, H)
                load_q6(whh2[d][:], OFF_WHH2 + d * 128 * PB,
                        SOFF_WHH2 + d * 128, H)
                for k in range(2):
                    load_q6(wxg2[d][k][:],
                            OFF_WXG2W + (d * 2 + k) * 128 * PB,
                            SOFF_WXG2W + (d * 2 + k) * 128, H)
                load_q8(wxg2c[d][:], OFF_WXG2B + d * 512, SOFF_WXG2B + d,
                        1, 4 * H)
            nc.sync.dma_start(hselt[:], d_hsel.ap())
            for g in range(8):
                nc.sync.dma_start(xidxt[16 * g:16 * (g + 1), :], d_xidx.ap())
            nc.sync.dma_start(wlin[:], d_wlin.ap())
            nc.sync.dma_start(blin[:], d_blin.ap())

            # identity matrix: ones masked to the diagonal (iota = col - p)
            nc.vector.memset(w_ident[:], 1.0)
            nc.gpsimd.affine_select(
                w_ident[:], w_ident[:], pattern=[[1, 128]],
                compare_op=ALU.is_ge, fill=0.0, base=0, channel_multiplier=-1)
            nc.gpsimd.affine_select(
                w_ident[:], w_ident[:], pattern=[[-1, 128]],
                compare_op=ALU.is_ge, fill=0.0, base=0, channel_multiplier=1)

            # invbc[p, t] = 1 / max(t, 1)
            iv32 = work.tile([128, T], mybir.dt.int32, tag="iv32")
            nc.gpsimd.iota(iv32[:], pattern=[[1, T]], base=0, channel_multiplier=0)
            nc.vector.tensor_copy(invbc[:], iv32[:])
            nc.vector.tensor_scalar_max(invbc[:], invbc[:], 1.0)
            nc.vector.reciprocal(invbc[:], invbc[:])

            # ---------------- phase A: embedding gather + xg1 ----------------
            nc.vector.memset(embT[:], 0.0)
            load_q8(embT[0:E + 1, :], OFF_EMBT, SOFF_EMBT, E + 1, V)
            for g in range(1, 8):
                nc.sync.dma_start(embT[16 * g:16 * g + E + 1, :],
                                  embT[0:E + 1, :])
            GCH = 512  # gather chunk (ISA dst-elem-count limit)
            for k in range((NBT + GCH - 1) // GCH):
                ch = min(GCH, NBT - k * GCH)
                nc.gpsimd.indirect_copy(
                    eT[:, k * GCH:k * GCH + ch], embT[:],
                    xidxt[:, k * GCH // 16:(k * GCH + ch) // 16], True)
            nc.vector.memset(ones1[:], 1.0)

            def xg_proj(lhsT_of, nk, rhs_of, evac_rev):
                """xg[, chunk m] = sum_k lhsT_k.T @ rhs_k ; evac (reversed for bwd)."""
                for m in range(G8):          # chunk (2g+d)
                    g, d = divmod(m, 2)
                    for b in range(BL):
                        ps = psb.tile([128, PSW], F32, tag="big")
                        for k in range(nk):
                            nc.tensor.matmul(
                                ps[:, 0:T], lhsT_of(d, g, k), rhs_of(d, k, b),
                                start=(k == 0), stop=(k == nk - 1),
                            )
                        dst = xg[:, m * NBT + b * T: m * NBT + (b + 1) * T]
                        if d == 1 and evac_rev:
                            dst = dst[:, ::-1]
                        nc.vector.tensor_copy(dst, ps[:, 0:T])

            # layer-1 projection: K = 11 (E rows + ones)
            xg_proj(
                lhsT_of=lambda d, g, k: (wxg1 if d == 0 else wxg1b)[:, g * H:(g + 1) * H],
                nk=1,
                rhs_of=lambda d, k, b: eT[0:E + 1, b * T:(b + 1) * T],
                evac_rev=True,
            )

            # ---------------- recurrence (both layers) ----------------
            def recurrence(whh, oTf, oTb):
                oTf_r = oTf[:].rearrange("p (b t) -> p b t", b=BL)
                oTb_r = oTb[:].rearrange("p (b t) -> p b t", b=BL)
                xg_r = xg[:].rearrange("p (c b t) -> p c b t", c=G8, b=BL)
                c_prev = None
                for t in range(T):
                    ps = psg.tile([128, 64], F32, tag="g")
                    nc.tensor.matmul(ps[:, 0:64], w_ident[:], xg_r[:, :, :, t],
                                     start=True, stop=(t == 0))
                    if t > 0:
                        for d in range(2):
                            tau = (t - 1) if d == 0 else (T - t)
                            h_ap = (oTf_r if d == 0 else oTb_r)[:, :, tau]
                            for g in range(4):
                                nc.tensor.matmul(
                                    ps[:, (2 * g + d) * BL:(2 * g + d + 1) * BL],
                                    whh[d][:, g * H:(g + 1) * H], h_ap,
                                    start=False, stop=(d == 1 and g == 3),
                                )
                    sig = work.tile([128, 64], F32, tag="sig")
                    nc.scalar.activation(sig[:, 0:48], ps[:, 0:48], AF.Sigmoid)
                    nc.scalar.activation(sig[:, 48:64], ps[:, 48:64], AF.Tanh)
                    cn = work.tile([128, 16], F32, tag="c")
                    if t > 0:
                        m1 = work.tile([128, 16], F32, tag="m1")
                        m2 = work.tile([128, 16], F32, tag="m2")
                        nc.vector.tensor_tensor(m2[:], sig[:, 0:16], sig[:, 48:64], ALU.mult)
                        nc.vector.tensor_tensor(m1[:], sig[:, 16:32], c_prev[:], ALU.mult)
                        nc.vector.tensor_tensor(cn[:], m1[:], m2[:], ALU.add)
                    else:
                        nc.vector.tensor_tensor(cn[:], sig[:, 0:16], sig[:, 48:64], ALU.mult)
                    th = work.tile([128, 16], F32, tag="th")
                    nc.scalar.activation(th[:], cn[:], AF.Tanh)
                    nc.vector.tensor_tensor(oTf_r[:, :, t], sig[:, 32:40], th[:, 0:8], ALU.mult)
                    nc.vector.tensor_tensor(oTb_r[:, :, T - 1 - t], sig[:, 40:48], th[:, 8:16], ALU.mult)
                    c_prev = cn

            recurrence(whh1, outTf, outTb)

            # ---------------- phase C: prefix + full attention (layer 1) -------
            outT = [outTf, outTb]
            # S/F: F[tc][:, b*T + i] = exp(sum_d out1[t',d] * out1[i,d] * inv(i))
            for b in range(BL):
                # per-batch scaled copies (rhs of the S matmul)
                outs_b = [work.tile([128, T], BF16, tag=f"outs{dc}", name=f"outs{dc}") for dc in range(2)]
                for dc in range(2):
                    nc.vector.tensor_tensor(
                        outs_b[dc][:], outT[dc][:, b * T:(b + 1) * T],
                        invbc[:], ALU.mult)
                for tcx in range(TC):
                    tch = min(128, T - tcx * 128)
                    ps = psb.tile([128, PSW], F32, tag="big")
                    for dc in range(2):
                        nc.tensor.matmul(
                            ps[0:tch, 0:T],
                            outT[dc][:, b * T + tcx * 128: b * T + tcx * 128 + tch],
                            outs_b[dc][:],
                            start=(dc == 0), stop=(dc == 1),
                        )
                    nc.scalar.activation(Fw[tcx][0:tch, b * T:(b + 1) * T],
                                         ps[0:tch, 0:T], AF.Exp)

            # local denominator sums over b: Dloc[:, tc*T + i]
            for tcx in range(TC):
                tch = min(128, T - tcx * 128)
                fr = Fw[tcx][0:tch].rearrange("p (b t) -> p t b", b=BL)
                nc.vector.tensor_reduce(
                    Dloc[0:tch, tcx * T:(tcx + 1) * T], fr,
                    axis=mybir.AxisListType.X, op=ALU.add)

            # h1 finals -> hb_in: cols 0:8 fwd (t=T-1), 8:16 bwd (t=0)
            oTf_r = outTf[:].rearrange("p (b t) -> p b t", b=BL)
            oTb_r = outTb[:].rearrange("p (b t) -> p b t", b=BL)
            hfin = work.tile([128, 16], BF16, tag="hfin")
            nc.vector.tensor_copy(hfin[:, 0:8], oTf_r[:, :, T - 1])
            nc.vector.tensor_copy(hfin[:, 8:16], oTb_r[:, :, 0])
            nc.sync.dma_start(hb_in[:], hfin[:])
            for tcx in range(TC):
                tch = min(128, T - tcx * 128)
                nc.sync.dma_start(db_in[tcx * 128:tcx * 128 + tch, :],
                                  Dloc[0:tch, tcx * T:(tcx + 1) * T])
            nc.gpsimd.collective_compute(
                "AllReduce", ALU.add, replica_groups=[list(range(NCORES))],
                ins=[db_in.opt()], outs=[db_out.opt()])
            nc.gpsimd.collective_compute(
                "AllGather", ALU.bypass, replica_groups=[list(range(NCORES))],
                ins=[hb_in.opt()], outs=[hb_out.opt()])
            for tcx in range(TC):
                tch = min(128, T - tcx * 128)
                nc.sync.dma_start(Dloc[0:tch, tcx * T:(tcx + 1) * T],
                                  db_out[tcx * 128:tcx * 128 + tch, :])
            nc.sync.dma_start(
                hgath[:], hb_out[:].rearrange("(c p) j -> p c j", p=128))

            # transpose out1T -> out1 [t, d] (per b, tc, dc), bf16
            for b in range(BL):
                for tcx in range(TC):
                    tch = min(128, T - tcx * 128)
                    for dc in range(2):
                        pt = psb.tile([128, PSW], BF16, tag="big")
                        nc.tensor.transpose(
                            pt[0:tch, 0:128],
                            outT[dc][:, b * T + tcx * 128: b * T + tcx * 128 + tch],
                            w_ident[:])
                        nc.vector.tensor_copy(
                            out1[0:tch, (b * TC + tcx) * 256 + dc * 128:
                                 (b * TC + tcx) * 256 + dc * 128 + 128],
                            pt[0:tch, 0:128])

            # reciprocal + strict lower-triangular mask on the denominators
            for tcx in range(TC):
                tch = min(128, T - tcx * 128)
                nc.vector.reciprocal(Dloc[0:tch, tcx * T:(tcx + 1) * T],
                                     Dloc[0:tch, tcx * T:(tcx + 1) * T])
                nc.gpsimd.affine_select(
                    Dloc[0:tch, tcx * T:(tcx + 1) * T],
                    Dloc[0:tch, tcx * T:(tcx + 1) * T],
                    pattern=[[1, T]], compare_op=ALU.is_gt, fill=0.0,
                    base=-tcx * 128, channel_multiplier=-1)

            # W~ = F * 1/D (masked), in place
            for tcx in range(TC):
                tch = min(128, T - tcx * 128)
                for b in range(BL):
                    nc.vector.tensor_tensor(
                        Fw[tcx][0:tch, b * T:(b + 1) * T],
                        Fw[tcx][0:tch, b * T:(b + 1) * T],
                        Dloc[0:tch, tcx * T:(tcx + 1) * T], ALU.mult)

            # att^T[dc][:, b*T + i] = sum_t out1[t, d] W~[t, i]
            for b in range(BL):
                for dc in range(2):
                    ps = psb.tile([128, PSW], F32, tag="big")
                    for tcx in range(TC):
                        tch = min(128, T - tcx * 128)
                        nc.tensor.matmul(
                            ps[:, 0:T],
                            out1[0:tch, (b * TC + tcx) * 256 + dc * 128:
                                 (b * TC + tcx) * 256 + dc * 128 + 128],
                            Fw[tcx][0:tch, b * T:(b + 1) * T],
                            start=(tcx == 0), stop=(tcx == TC - 1),
                        )
                    nc.vector.tensor_copy(attT[dc][:, b * T:(b + 1) * T], ps[:, 0:T])

            # ---- full attention #1 (scores vs torch-reshaped h_n) ----
            def full_attention(oT_pair, out_sb, attdst):
                """scores from oT_pair lhsT + hid rhs; writes at into `at1`;
                returns after computing a2 columns into attdst (list per dc)."""
                nc.gpsimd.indirect_copy(hid[:], hgath[:], hselt[:], True)
                sc = pss.tile([128, TC * BL], F32, tag="small")
                if T % 128 != 0:
                    nc.vector.memset(sc[:], 0.0)
                for b in range(BL):
                    for tcx in range(TC):
                        tch = min(128, T - tcx * 128)
                        for dc in range(2):
                            nc.tensor.matmul(
                                sc[0:tch, tcx * BL + b: tcx * BL + b + 1],
                                oT_pair[dc][:, b * T + tcx * 128: b * T + tcx * 128 + tch],
                                hid[:, 2 * b + dc: 2 * b + dc + 1],
                                start=(dc == 0), stop=(dc == 1),
                            )
                nc.scalar.activation(ate[:], sc[:], AF.Exp, scale=1.0 / T)
                ar = ate[:].rearrange("p (t b) -> p t b", b=BL)
                nc.vector.tensor_reduce(dloc_s[:], ar, axis=mybir.AxisListType.X,
                                        op=ALU.add)
                return sc

            sc1 = full_attention([outTf, outTb], out1, attT)
            nc.sync.dma_start(sb_in[:], dloc_s[:])
            nc.gpsimd.collective_compute(
                "AllReduce", ALU.add, replica_groups=[list(range(NCORES))],
                ins=[sb_in.opt()], outs=[sb_out.opt()])
            nc.sync.dma_start(drec_s[:], sb_out[:])
            nc.vector.reciprocal(drec_s[:], drec_s[:])

            def finish_attention(attdst, col):
                """at = ate/d ; a2^T[dc] = sum_t out1[t,d] at[t] -> attdst[dc][:, col+b*T]"""
                for b in range(BL):
                    nc.vector.tensor_tensor(
                        at1[:].rearrange("p (t b) -> p t b", b=BL)[:, :, b],
                        ate[:].rearrange("p (t b) -> p t b", b=BL)[:, :, b],
                        drec_s[:], ALU.mult)
                for b in range(BL):
                    for dc in range(2):
                        pa = pss.tile([128, TC * BL], F32, tag="small")
                        for tcx in range(TC):
                            tch = min(128, T - tcx * 128)
                            nc.tensor.matmul(
                                pa[0:128, 0:1],
                                out1[0:tch, (b * TC + tcx) * 256 + dc * 128:
                                     (b * TC + tcx) * 256 + dc * 128 + 128],
                                at1[0:tch, tcx * BL + b: tcx * BL + b + 1],
                                start=(tcx == 0), stop=(tcx == TC - 1),
                            )
                        if attdst is not None:
                            nc.vector.tensor_copy(
                                attdst[dc][:, b * T + col: b * T + col + 1],
                                pa[:, 0:1])
                        else:
                            nc.vector.tensor_copy(
                                a2sb[:, b * 2 + dc: b * 2 + dc + 1], pa[:, 0:1])

            finish_attention(attT, T - 1)

            # ---------------- phase D: xg2 projection ----------------
            rhs2 = [attT[0], attT[1]]
            for m in range(G8):
                g, d = divmod(m, 2)
                for b in range(BL):
                    ps = psb.tile([128, PSW], F32, tag="big")
                    for k in range(2):
                        nc.tensor.matmul(
                            ps[:, 0:T], wxg2[d][k][:, g * H:(g + 1) * H],
                            rhs2[k][:, b * T:(b + 1) * T],
                            start=(k == 0), stop=False)
                    nc.tensor.matmul(
                        ps[:, 0:T], wxg2c[d][:, g * H:(g + 1) * H],
                        ones1[:],
                        start=False, stop=True)
                    dst = xg[:, m * NBT + b * T: m * NBT + (b + 1) * T]
                    if d == 1:
                        dst = dst[:, ::-1]
                    nc.vector.tensor_copy(dst, ps[:, 0:T])

            # ---------------- phase E: layer-2 recurrence ----------------
            if debug:
                nc.sync.dma_start(d_out1Tf.ap(), outTf[:])
                nc.sync.dma_start(d_out1Tb.ap(), outTb[:])
                nc.sync.dma_start(d_attT0.ap(), attT[0][:])
                nc.sync.dma_start(d_attT1.ap(), attT[1][:])
            recurrence(whh2, outTf, outTb)
            if debug:
                nc.sync.dma_start(d_out2Tf.ap(), outTf[:])
                nc.sync.dma_start(d_out2Tb.ap(), outTb[:])

            # ---------------- phase F: final full attention + linear ----------
            # h2 finals gather
            hfin2 = work.tile([128, 16], BF16, tag="hfin")
            nc.vector.tensor_copy(hfin2[:, 0:8], oTf_r[:, :, T - 1])
            nc.vector.tensor_copy(hfin2[:, 8:16], oTb_r[:, :, 0])
            nc.sync.dma_start(hb2_in[:], hfin2[:])
            nc.gpsimd.collective_compute(
                "AllGather", ALU.bypass, replica_groups=[list(range(NCORES))],
                ins=[hb2_in.opt()], outs=[hb2_out.opt()])
            nc.sync.dma_start(
                hgath[:], hb2_out[:].rearrange("(c p) j -> p c j", p=128))

            # transpose out2T -> out1 buffer ([t, d] layout)
            for b in range(BL):
                for tcx in range(TC):
                    tch = min(128, T - tcx * 128)
                    for dc in range(2):
                        pt = psb.tile([128, PSW], BF16, tag="big")
                        nc.tensor.transpose(
                            pt[0:tch, 0:128],
                            outT[dc][:, b * T + tcx * 128: b * T + tcx * 128 + tch],
                            w_ident[:])
                        nc.vector.tensor_copy(
                            out1[0:tch, (b * TC + tcx) * 256 + dc * 128:
                                 (b * TC + tcx) * 256 + dc * 128 + 128],
                            pt[0:tch, 0:128])

            sc2 = full_attention([outTf, outTb], out1, None)
            nc.sync.dma_start(sb2_in[:], dloc_s[:])
            nc.gpsimd.collective_compute(
                "AllReduce", ALU.add, replica_groups=[list(range(NCORES))],
                ins=[sb2_in.opt()], outs=[sb2_out.opt()])
            nc.sync.dma_start(drec_s[:], sb2_out[:])
            nc.vector.reciprocal(drec_s[:], drec_s[:])
            finish_attention(None, 0)

            # y = sigmoid(a2 @ w + b)
            py = pss.tile([128, TC * BL], F32, tag="small")
            a2r = a2sb[:].rearrange("p (b k) -> p b k", k=2)
            for dc in range(2):
                nc.tensor.matmul(py[0:1, 0:BL], wlin[:, dc:dc + 1], a2r[:, :, dc],
                                 start=(dc == 0), stop=(dc == 1))
            nc.scalar.activation(ysb[:], py[0:1, 0:BL], AF.Sigmoid, bias=blin[:])
            nc.sync.dma_start(d_y.ap(), ysb[:])

    nc.compile()
    return nc


# ======================= host-side wrapper =======================

def _q8_rows(w):
    """[R, C] f32 -> (int8 [R, C], f32 scale [R]) symmetric per-row quant."""
    w = np.asarray(w, np.float32)
    s = np.abs(w).max(axis=1) / 127.0
    s = np.where(s == 0, 1.0, s).astype(np.float32)
    q = np.round(w / s[:, None]).clip(-127, 127).astype(np.int8)
    return q, s


def _q6_rows(w):
    """[R, C] f32 -> (u8-packed [R, 3C/4] as int8, f32 scale [R]).

    Symmetric per-row int6: q in [-31, 31], stored biased (+32) as 6-bit
    values, 4 values packed little-endian into 3 bytes."""
    w = np.asarray(w, np.float32)
    s = np.abs(w).max(axis=1) / 31.0
    s = np.where(s == 0, 1.0, s).astype(np.float32)
    v = (np.round(w / s[:, None]).clip(-31, 31).astype(np.int32) + 32)
    v4 = v.reshape(v.shape[0], -1, 4)
    b0 = (v4[:, :, 0] | (v4[:, :, 1] << 6)) & 255
    b1 = ((v4[:, :, 1] >> 2) | (v4[:, :, 2] << 4)) & 255
    b2 = ((v4[:, :, 2] >> 4) | (v4[:, :, 3] << 2)) & 255
    packed = np.stack([b0, b1, b2], axis=-1).reshape(v.shape[0], -1)
    return packed.astype(np.uint8).view(np.int8), s


GATE_PERM = [0, 1, 3, 2]  # torch (i,f,g,o) chunks -> ours (i,f,o,g)


def _reorder_gates(w):
    """w [4H, ...] in torch gate order -> [4H, ...] in (i,f,o,g) order."""
    chunks = [w[g * H:(g + 1) * H] for g in GATE_PERM]
    return np.concatenate(chunks, axis=0)


def _pack_xgw(Wih, bih, bhh):
    """-> [K+1, 4H] rows: Wih^T then combined bias row (gate-reordered)."""
    Wr = _reorder_gates(np.asarray(Wih))          # [4H, K]
    br = _reorder_gates((np.asarray(bih) + np.asarray(bhh))[:, None])[:, 0]  # [4H]
    return np.concatenate([Wr.T, br[None, :]], axis=0)  # [K+1, 4H]


def _pack_whh(Whh):
    return _reorder_gates(np.asarray(Whh)).T      # [H, 4H]


def _wrap16(flat):
    """flat [N] -> [16, N//16] wrapped (s p) per 16-group (replicated on device)."""
    return np.ascontiguousarray(flat.reshape(-1, 16).T)  # [16, s]


def prepare_inputs(T, x, emb, l1_Wih_f, l1_Whh_f, l1_bih_f, l1_bhh_f,
                   l1_Wih_b, l1_Whh_b, l1_bih_b, l1_bhh_b,
                   l2_Wih_f, l2_Whh_f, l2_bih_f, l2_bhh_f,
                   l2_Wih_b, l2_Whh_b, l2_bih_b, l2_bhh_b, lin_W, lin_b):
    """Build per-core in_maps."""
    x = np.asarray(x).astype(np.int64)
    embT = np.concatenate(
        [np.asarray(emb, np.float32).T, np.ones((1, V), np.float32)], axis=0)
    w2f = _pack_xgw(l2_Wih_f, l2_bih_f, l2_bhh_f)   # [257, 512]
    w2b = _pack_xgw(l2_Wih_b, l2_bih_b, l2_bhh_b)
    qs = [
        _q8_rows(_pack_xgw(l1_Wih_f, l1_bih_f, l1_bhh_f)),
        _q8_rows(_pack_xgw(l1_Wih_b, l1_bih_b, l1_bhh_b)),
        _q6_rows(_pack_whh(l1_Whh_f)),
        _q6_rows(_pack_whh(l1_Whh_b)),
        _q6_rows(w2f[0:128]), _q6_rows(w2f[128:256]),
        _q6_rows(w2b[0:128]), _q6_rows(w2b[128:256]),
        _q8_rows(w2f[256:257]), _q8_rows(w2b[256:257]),
        _q6_rows(_pack_whh(l2_Whh_f)),
        _q6_rows(_pack_whh(l2_Whh_b)),
        _q8_rows(embT),
    ]
    blob = np.concatenate([q.ravel() for q, _ in qs])
    scales = np.concatenate([s for _, s in qs])
    assert blob.shape[0] == WBLOB_LEN and scales.shape[0] == SLEN
    blob = np.concatenate(
        [blob, np.zeros(NCORES * WSH - WBLOB_LEN, dtype=np.int8)])
    scales = np.concatenate(
        [scales, np.zeros(NCORES * SSH - SLEN, dtype=np.float32)])
    wshards = blob.reshape(NCORES, WSH)
    sshards = scales.reshape(NCORES, SSH)
    shared = {
        "wlin": np.asarray(lin_W, dtype=np.float32).reshape(256)
                  .reshape(2, 128).T.copy(),
        "blin": np.asarray(lin_b, dtype=np.float32).reshape(1, 1),
    }
    in_maps = []
    for c in range(NCORES):
        xl = x[c * BL:(c + 1) * BL, :]            # [BL, T]
        xflat = xl.reshape(-1).astype(np.uint16)  # b-major
        # hidden-selection gather indices for this core (torch h_n reshape)
        L = np.zeros(16, dtype=np.uint16)
        for bl in range(BL):
            bglob = c * BL + bl
            for k in range(2):
                if bglob < B // 2:
                    gidx = 2 * bglob + k
                    col = (gidx // BL) * 16 + (gidx % BL)
                else:
                    gidx = 2 * bglob - B + k
                    col = (gidx // BL) * 16 + 8 + (gidx % BL)
                L[2 * bl + k] = col
        hsel = np.zeros((128, 1), dtype=np.uint16)
        for g in range(8):
            hsel[16 * g:16 * (g + 1), 0] = L
        m = dict(shared)
        m["wblob"] = wshards[c]
        m["wscale"] = sshards[c]
        m["xidx"] = _wrap16(xflat)
        m["hsel"] = hsel
        in_maps.append(m)
    return in_maps


_CACHE = {}
_RUN_CACHE = {}


def _make_runner(nc, n_cores=NCORES):
    """Build a cached jitted PJRT runner (mirrors bass2jax.run_bass_via_pjrt)."""
    import jax
    from jax.experimental.shard_map import shard_map
    from jax.sharding import Mesh, PartitionSpec
    from concourse import bass2jax

    bass2jax.install_neuronx_cc_hook()
    partition_name = (nc.partition_id_tensor.name
                      if nc.partition_id_tensor else None)
    in_names, out_names, out_avals, zero_shapes = [], [], [], []
    for alloc in nc.m.functions[0].allocations:
        if not isinstance(alloc, mybir.MemoryLocationSet):
            continue
        name = alloc.memorylocations[0].name
        if alloc.kind == "ExternalInput":
            if name != partition_name:
                in_names.append(name)
        elif alloc.kind == "ExternalOutput":
            shape = tuple(alloc.tensor_shape)
            dtype = mybir.dt.np(alloc.dtype)
            out_names.append(name)
            out_avals.append(jax.core.ShapedArray(shape, dtype))
            zero_shapes.append((shape, dtype))
    n_params = len(in_names)
    n_outs = len(out_avals)
    all_names = in_names + out_names
    if partition_name is not None:
        all_names.append(partition_name)
    donate = tuple(range(n_params, n_params + n_outs))

    def _body(*args):
        operands = list(args)
        if partition_name is not None:
            operands.append(bass2jax.partition_id_tensor())
        outs = bass2jax._bass_exec_p.bind(
            *operands, out_avals=tuple(out_avals), in_names=tuple(all_names),
            out_names=tuple(out_names), lowering_input_output_aliases=(),
            sim_require_finite=False, sim_require_nnan=False, nc=nc)
        return tuple(outs)

    devices = jax.devices()[:n_cores]
    mesh = Mesh(np.asarray(devices), ("core",))
    in_specs = (PartitionSpec("core"),) * (n_params + n_outs)
    out_specs = (PartitionSpec("core"),) * n_outs
    sharded = jax.jit(
        shard_map(_body, mesh=mesh, in_specs=in_specs, out_specs=out_specs,
                  check_rep=False),
        donate_argnums=donate, keep_unused=True)

    def runner(in_maps):
        per_core = [[np.asarray(m[n]) for n in in_names] for m in in_maps]
        concat_in = [np.concatenate([per_core[c][i] for c in range(n_cores)],
                                    axis=0) for i in range(n_params)]
        concat_zeros = [np.zeros((n_cores * s[0], *s[1:]), d)
                        for (s, d) in zero_shapes]
        out_arrs = sharded(*concat_in, *concat_zeros)
        out_arrs = [np.asarray(a) for a in out_arrs]
        return [
            {name: out_arrs[i].reshape(n_cores, *out_avals[i].shape)[c]
             for i, name in enumerate(out_names)}
            for c in range(n_cores)]

    return runner


def get_runner(T, debug=False):
    key = (T, debug)
    if key not in _RUN_CACHE:
        _RUN_CACHE[key] = _make_runner(_get_program(T, debug))
    return _RUN_CACHE[key]


def _get_program(T, debug):
    key = (T, debug)
    if key not in _CACHE:
        _CACHE[key] = build_program(T, debug)
    return _CACHE[key]


def run(T, inputs, debug=False, trace=False):
    nc = _get_program(T, debug)
    in_maps = prepare_inputs(T, **inputs)
    res = bass_utils.run_bass_kernel_spmd(
        nc, in_maps, core_ids=list(range(NCORES)), trace=trace)
    y = np.concatenate([res.results[c]["y"].reshape(BL) for c in range(NCORES)])
    return y.reshape(B, 1).astype(np.float32), res


def kernel(**inputs) -> np.ndarray:
    T = np.asarray(inputs["x"]).shape[1]
    try:
        y, _ = run(T, inputs, debug=False, trace=False)
    except Exception:
        # transient NRT/device errors (e.g. NRT_EXEC_UNIT_UNRECOVERABLE)
        # usually clear on retry
        import time
        time.sleep(15)
        y, _ = run(T, inputs, debug=False, trace=False)
    return y

